# revision 21
# baseline (speedup 1.0000x reference)
"""Trainium2 Bass kernel for nn_MAB_65068754534455 (dense transformer MAB block).

Computation (per reference):
  q = query @ Wq.T + bq ; k = kv @ Wk.T + bk ; v = kv @ Wv.T + bv
  per head: A = softmax(q k^T / sqrt(hd)) ; o = A v
  x = qheads + o (merged) ; out = x + relu(x @ Wo.T + bo)

Sharding: 8 cores = 4 batches x 2 query-halves (data parallel, no collectives).
Each core computes K/V projections for its batch (duplicated across the pair)
and attention + output projection for its 1024 query rows.

On-chip layout is feature-major ("transposed"): activations live as X^T [d, t]
so every matmul contraction dim sits on partitions with zero on-device
transposes. The host pre-transposes inputs/weights (not timed as HW).

Engine choreography (v3):
 - Q projection + scores bf16; K/V projections fp8e4 DoubleRow (2 virtual
   k-tiles per matmul); PV matmuls fp8 DoubleRow over key-tile pairs
   (A in fp8e5, V in fp8e4) - attention weights tolerate fp8 since softmax
   averages ~2048 near-uniform terms
 - softmax exp split across engines: ScalarE table exp -> fp8e5; VectorE
   Schraudolph fast-exp (x*A+B -> int8 -> bitcast fp8e5) on a tunable share
 - score matmuls for a head pair (64-wide contraction) run concurrently via
   tile_position row packing
 - softmax denominators from a ones-column in V (PSUM row 64); reciprocal via
   reciprocal_approx_fast on an [8,128] reshaped strip; normalization
   mult/adds load-balanced over Vector/GpSimd via nc.any
 - qb-outer loop: the output projection of query-block 0 is emitted
   interleaved into query-block 1's attention
"""

import math

import numpy as np
import ml_dtypes

import concourse.mybir as mybir
import concourse.tile as tile
from concourse import bacc
from concourse.bass_utils import run_bass_kernel_spmd

# problem constants (hardcoded per spec)
B, SQ, SKV, D, H = 4, 2048, 2048, 512, 8
HD = D // H                      # 64
SCALE = 1.0 / math.sqrt(HD)
NCORES = 8
TQ = SQ // 2                     # 1024 query rows per core

F32 = mybir.dt.float32
BF16 = mybir.dt.bfloat16
I8 = mybir.dt.int8
F8E4 = mybir.dt.float8e4
F8E5 = mybir.dt.float8e5

KT = D // 128                    # 4 contraction k-tiles
DT = D // 128                    # 4 output d-tiles (= head pairs)
NQB = TQ // 512                  # 2 query blocks of 512
NKB = SKV // 512                 # 4 key blocks of 512
NTK = SKV // 128                 # 16 key tiles of 128
NTM = NTK // 2                   # 8 key-tile pairs (fp8 DoubleRow)
VW = HD + 1                      # 65: V head block width incl. ones column
VWP = 80                         # padded head block (DoubleRow pair stride %16)

LOG2E = 1.4426950408889634
A8 = 4.0 * LOG2E * SCALE         # fast-exp slope for e5m2 bits (SCALE folded)
B8 = 60.0 - 486411.0 / (2 ** 21)  # fast-exp bias (round-to-nearest)

# exp engine assignment: strict alternation - each DoubleRow key-tile pair
# gets one ACT tile and one DVE tile running concurrently
def _dve_tile(gidx: int) -> bool:
    return gidx % 2 == 1


AF = mybir.ActivationFunctionType
OP = mybir.AluOpType
DR = mybir.MatmulPerfMode.DoubleRow


def _build():
    nc = bacc.Bacc(None, target_bir_lowering=False, debug=False)

    xqt = nc.dram_tensor("xqt", [D, TQ], BF16, kind="ExternalInput").ap()
    xkv8 = nc.dram_tensor("xkv8", [128, 2, 2, SKV], F8E4, kind="ExternalInput").ap()
    wqt = nc.dram_tensor("wqt", [D, D], BF16, kind="ExternalInput").ap()
    wk8 = nc.dram_tensor("wk8", [128, 2, 2, D], F8E4, kind="ExternalInput").ap()
    wv8 = nc.dram_tensor("wv8", [128, 2, 2, D], F8E4, kind="ExternalInput").ap()
    wot = nc.dram_tensor("wot", [D, D], BF16, kind="ExternalInput").ap()
    biasc = nc.dram_tensor("biasc", [128, 3 * DT + D], F32, kind="ExternalInput").ap()
    outt = nc.dram_tensor("outt", [D, TQ], BF16, kind="ExternalOutput").ap()

    with tile.TileContext(nc) as tc:
        with (
            tc.tile_pool(name="persist", bufs=1) as pp,
            tc.tile_pool(name="ps", bufs=1, space="PSUM") as psp,
            tc.tile_pool(name="e2p", bufs=4) as ep,
            tc.tile_pool(name="ocp", bufs=3) as ocp,
            tc.tile_pool(name="onp", bufs=3) as onp,
            tc.tile_pool(name="on64p", bufs=3) as on64p,
            tc.tile_pool(name="rbcp", bufs=3) as rbcp,
            tc.tile_pool(name="ytp", bufs=3) as ytp,
        ):
            w_q = pp.tile([128, KT, D], BF16)
            w_k8 = pp.tile([128, 2, 2, D], F8E4)
            w_v8 = pp.tile([128, 2, 2, D], F8E4)
            w_o = pp.tile([128, KT, D], BF16)
            qt = pp.tile([128, DT, TQ], BF16)      # Q^T, becomes x^T
            kt = pp.tile([128, DT, SKV], BF16)     # K^T (scores lhsT)
            # V as fp8, key-tile pairs: [p, m, j, head*VWP + d] (ones col at 64)
            v8 = pp.tile([128, NTM, 2, H * VWP], F8E4)
            xq_s = pp.tile([128, KT, TQ], BF16)
            xkv_s = pp.tile([128, 2, 2, SKV], F8E4)
            bias_s = pp.tile([128, 3 * DT + D], F32)
            bq_s = bias_s[:, 0:DT]
            bk_s = bias_s[:, DT : 2 * DT]
            bo_s = bias_s[:, 2 * DT : 3 * DT]
            bv_s = bias_s[:, 3 * DT :]

            # ---- input DMA: biases + K-projection path first ----
            nc.sync.dma_start(bias_s[:], biasc)
            nc.sync.dma_start(w_k8[:], wk8)
            nc.sync.dma_start(xkv_s[:], xkv8)
            wq_r = wqt.rearrange("(o p) d -> p o d", p=128)
            xq_r = xqt.rearrange("(o p) t -> p o t", p=128)
            for k in range(KT):
                nc.sync.dma_start(w_q[:, k, :], wq_r[:, k, :])
                nc.sync.dma_start(xq_s[:, k, :], xq_r[:, k, :])
            nc.sync.dma_start(w_v8[:], wv8)
            nc.sync.dma_start(w_o[:], wot.rearrange("(o p) d -> p o d", p=128))

            # ones columns of V (col 64 of each VWP-wide head block)
            for m in range(NTM):
                for j in range(2):
                    nc.any.memset(
                        v8[:, m, j, :].rearrange("p (h w) -> p h w", w=VWP)[:, :, HD],
                        1.0,
                    )

            # ---------------- phase 1: projections ----------------
            def q_proj(j, q):
                # one 512-wide q block; evac per block (bias via ACT)
                ps = psp.tile([128, 2, 512], F32, tag="s2", bufs=3, name="pjq")
                for k in range(KT):
                    nc.tensor.matmul(
                        ps[:, 0, :], w_q[:, k, j * 128 : (j + 1) * 128],
                        xq_s[:, k, q * 512 : (q + 1) * 512],
                        start=(k == 0), stop=(k == KT - 1),
                    )
                nc.scalar.activation(
                    qt[:, j, q * 512 : (q + 1) * 512], ps[:, 0, :],
                    AF.Identity, bias=bq_s[:, j : j + 1],
                )

            def k_proj2(j, qp):
                # two 512-wide key blocks into one 2-bank tile, single evac
                ps = psp.tile([128, 2, 512], F32, tag="s2", bufs=3, name="pjk")
                for q in range(2):
                    for kp in range(2):
                        nc.tensor.matmul(
                            ps[:, q, :], w_k8[:, kp, :, j * 128 : (j + 1) * 128],
                            xkv_s[:, kp, :, (2 * qp + q) * 512 : (2 * qp + q + 1) * 512],
                            start=(kp == 0), stop=(kp == 1), perf_mode=DR,
                        )
                nc.scalar.activation(
                    kt[:, j, qp * 1024 : (qp + 1) * 1024],
                    ps[:].rearrange("p a b -> p (a b)"),
                    AF.Identity, bias=bk_s[:, j : j + 1],
                )

            def v_proj(i):
                ps = psp.tile([128, 2, 512], F32, tag="s2", bufs=3, name="pjv")
                for kp in range(2):
                    nc.tensor.matmul(
                        ps[:, 0, :], xkv_s[:, kp, :, i * 128 : (i + 1) * 128],
                        w_v8[:, kp, :, :],
                        start=(kp == 0), stop=(kp == 1), perf_mode=DR,
                    )
                nc.vector.tensor_tensor(
                    v8[:, i // 2, i % 2, :]
                    .rearrange("p (h w) -> p h w", w=VWP)[:, :, 0:HD],
                    ps[:, 0, :].rearrange("p (h w) -> p h w", w=HD),
                    bv_s.rearrange("p (h w) -> p h w", w=HD),
                    OP.add,
                )

            # ---------------- phases 1+2+3 interleaved, qb-outer ----------------
            def phase3(qb, j):
                qsl = slice(qb * 512, (qb + 1) * 512)
                z = psp.tile([128, 2, 512], F32, tag="s2", bufs=3, name="zt")
                for k in range(KT):
                    nc.tensor.matmul(
                        z[:, 0, :], w_o[:, k, j * 128 : (j + 1) * 128],
                        qt[:, k, qsl],
                        start=(k == 0), stop=(k == KT - 1),
                    )
                yt = ytp.tile([128, 512], F32)
                nc.scalar.activation(
                    yt[:], z[:, 0, :], AF.Relu, bias=bo_s[:, j : j + 1]
                )
                yt2 = ytp.tile([128, 512], BF16, tag="y2", name="yt2")
                nc.any.tensor_tensor(yt2[:], yt[:], qt[:, j, qsl], OP.add)
                nc.sync.dma_start(outt[j * 128 : (j + 1) * 128, qsl], yt2[:])

            for qb in range(NQB):
                qsl = slice(qb * 512, (qb + 1) * 512)
                for hp in range(DT):
                    if qb == 0:
                        # emit this head-pair's projections just ahead of its
                        # attention; the scheduler fills PE gaps with them
                        q_proj(hp, 0)
                        for qp in range(2):
                            k_proj2(hp, qp)
                        for i in range(4 * hp, 4 * hp + 4):
                            v_proj(i)
                        if hp == DT - 1:
                            for j in range(DT):
                                q_proj(j, 1)
                    h_e, h_o = 2 * hp, 2 * hp + 1
                    o2 = psp.tile([65, 2, 512], F32, tag="o", bufs=1, name="o2")
                    for m in range(NTM):
                        # e2 pair-tile: [p, j(key subtile), parity, q] fp8e5
                        e2 = ep.tile([128, 2, 2, 512], F8E5)
                        for jj in range(2):
                            i = 2 * m + jj
                            gidx = (qb * DT + hp) * NTK + i
                            isl = slice(i * 128, (i + 1) * 128)
                            s2 = psp.tile(
                                [128, 2, 512], F32, tag="s2", bufs=3, name="s2t"
                            )
                            nc.tensor.matmul(
                                s2[:, 0, :], kt[0:64, hp, isl], qt[0:64, hp, qsl],
                                start=True, stop=True, tile_position=(0, 0),
                            )
                            nc.tensor.matmul(
                                s2[:, 1, :], kt[64:128, hp, isl],
                                qt[64:128, hp, qsl],
                                start=True, stop=True, tile_position=(64, 0),
                            )
                            if _dve_tile(gidx):
                                nc.vector.tensor_scalar(
                                    e2[:, jj, :, :]
                                    .rearrange("p a b -> p (a b)").bitcast(I8),
                                    s2[:].rearrange("p a b -> p (a b)"),
                                    A8, B8, OP.mult, OP.add,
                                )
                            else:
                                nc.scalar.activation(
                                    e2[:, jj, :, :].rearrange("p a b -> p (a b)"),
                                    s2[:].rearrange("p a b -> p (a b)"),
                                    AF.Exp, scale=SCALE,
                                )
                        for p, h in ((0, h_e), (1, h_o)):
                            nc.tensor.matmul(
                                o2[:, p, :],
                                v8[:, m, :, h * VWP : h * VWP + VW],
                                e2[:, :, p, :],
                                start=(m == 0), stop=(m == NTM - 1),
                                perf_mode=DR,
                            )

                    # evacuate o + r; normalize; residual-add into qt
                    oc = ocp.tile([65, 2, 512], F32, name="oct")
                    nc.scalar.activation(oc[:, 0, :], o2[:, 0, :], AF.Copy)
                    nc.vector.tensor_copy(oc[:, 1, :], o2[:, 1, :])
                    # r rows ([1,1024] f32) -> [8,128] tile so the reciprocal
                    # runs 8 lanes wide (DMA reshapes by linearization)
                    rcol = rbcp.tile([8, 128], F32, tag="rc", name="rcol")
                    nc.sync.dma_start(rcol[:], oc[64:65, :, :])
                    rinv = rbcp.tile([8, 128], F32, tag="ri", name="rinv")
                    nc.vector.reciprocal_approx_fast(rinv[:], rcol[:])
                    rflat = rbcp.tile([1, 2, 512], F32, tag="rf", name="rflat")
                    nc.sync.dma_start(rflat[:], rinv[:])
                    rbc = rbcp.tile([64, 2, 512], F32, tag="rb", name="rbc")
                    nc.gpsimd.partition_broadcast(rbc[:, 0, :], rflat[0:1, 0, :])
                    nc.gpsimd.partition_broadcast(rbc[:, 1, :], rflat[0:1, 1, :])
                    on = onp.tile([64, 2, 512], BF16)
                    nc.any.tensor_tensor(on[:], oc[0:64, :, :], rbc[:], OP.mult)
                    nc.any.tensor_tensor(
                        qt[0:64, hp, qsl], qt[0:64, hp, qsl], on[:, 0, :], OP.add
                    )
                    on64 = on64p.tile([128, 512], BF16)
                    nc.sync.dma_start(on64[64:128, :], on[:, 1, :])
                    nc.any.tensor_tensor(
                        qt[64:128, hp, qsl], qt[64:128, hp, qsl],
                        on64[64:128, :], OP.add,
                    )
                    if qb == 1:
                        # overlap qb0's output projection with qb1's attention
                        phase3(0, hp)

            for j in range(DT):
                phase3(1, j)

    nc.compile()
    return nc


_NC = None


def _get_nc():
    global _NC
    if _NC is None:
        _NC = _build()
    return _NC


def _interleave8(a):
    """[512, N] f32 -> [128, 2, 2, N] fp8e4 with row (kp*256 + j*128 + p) at
    [p, kp, j]."""
    f8 = ml_dtypes.float8_e4m3
    return np.ascontiguousarray(
        a.reshape(2, 2, 128, a.shape[1]).transpose(2, 0, 1, 3)
    ).astype(f8)


def kernel(**inputs) -> np.ndarray:
    bf = ml_dtypes.bfloat16
    q = np.asarray(inputs["query"], dtype=np.float32)
    kv = np.asarray(inputs["key_value"], dtype=np.float32)
    shared = {
        "wqt": np.ascontiguousarray(np.asarray(inputs["Wq"], np.float32).T).astype(bf),
        "wk8": _interleave8(np.asarray(inputs["Wk"], np.float32).T),
        "wv8": _interleave8(np.asarray(inputs["Wv"], np.float32).T),
        "wot": np.ascontiguousarray(np.asarray(inputs["Wo"], np.float32).T).astype(bf),
        "biasc": np.ascontiguousarray(np.concatenate(
            [
                np.asarray(inputs["bq"], np.float32).reshape(DT, 128).T,
                np.asarray(inputs["bk"], np.float32).reshape(DT, 128).T,
                np.asarray(inputs["bo"], np.float32).reshape(DT, 128).T,
                np.broadcast_to(np.asarray(inputs["bv"], np.float32), (128, D)),
            ],
            axis=1,
        )),
    }
    in_maps = []
    for c in range(NCORES):
        b, half = divmod(c, 2)
        qs = q[b, half * TQ : (half + 1) * TQ]
        in_maps.append(
            {
                "xqt": np.ascontiguousarray(qs.T).astype(bf),
                "xkv8": _interleave8(np.ascontiguousarray(kv[b].T)),
                **shared,
            }
        )

    nc = _get_nc()
    res = run_bass_kernel_spmd(nc, in_maps, core_ids=list(range(NCORES)))
    kernel._last_results = res  # for test harness introspection

    out = np.empty((B, SQ, D), np.float32)
    for c in range(NCORES):
        b, half = divmod(c, 2)
        out[b, half * TQ : (half + 1) * TQ] = res.results[c]["outt"].astype(np.float32).T
    return out


# revision 22
# speedup vs baseline: 1.1905x; 1.1905x over previous
"""Trainium2 Bass kernel for nn_MAB_65068754534455 (dense transformer MAB block).

Computation (per reference):
  q = query @ Wq.T + bq ; k = kv @ Wk.T + bk ; v = kv @ Wv.T + bv
  per head: A = softmax(q k^T / sqrt(hd)) ; o = A v
  x = qheads + o (merged) ; out = x + relu(x @ Wo.T + bo)

Sharding: 8 cores = 4 batches x 2 query-halves (data parallel, no collectives).
Each core computes K/V projections for its batch (duplicated across the pair)
and attention + output projection for its 1024 query rows.

On-chip layout is feature-major ("transposed"): activations live as X^T [d, t]
so every matmul contraction dim sits on partitions with zero on-device
transposes. The host pre-transposes inputs/weights (not timed as HW).

Engine choreography (v3):
 - Q projection + scores bf16; K/V projections fp8e4 DoubleRow (2 virtual
   k-tiles per matmul); PV matmuls fp8 DoubleRow over key-tile pairs
   (A in fp8e5, V in fp8e4) - attention weights tolerate fp8 since softmax
   averages ~2048 near-uniform terms
 - softmax exp split across engines: ScalarE table exp -> fp8e5; VectorE
   Schraudolph fast-exp (x*A+B -> int8 -> bitcast fp8e5) on a tunable share
 - score matmuls for a head pair (64-wide contraction) run concurrently via
   tile_position row packing
 - softmax denominators from a ones-column in V (PSUM row 64); reciprocal via
   reciprocal_approx_fast on an [8,128] reshaped strip; normalization
   mult/adds load-balanced over Vector/GpSimd via nc.any
 - qb-outer loop: the output projection of query-block 0 is emitted
   interleaved into query-block 1's attention
"""

import math

import numpy as np
import ml_dtypes

import concourse.mybir as mybir
import concourse.tile as tile
from concourse import bacc
from concourse.bass_utils import run_bass_kernel_spmd

# problem constants (hardcoded per spec)
B, SQ, SKV, D, H = 4, 2048, 2048, 512, 8
HD = D // H                      # 64
SCALE = 1.0 / math.sqrt(HD)
NCORES = 8
TQ = SQ // 2                     # 1024 query rows per core

F32 = mybir.dt.float32
BF16 = mybir.dt.bfloat16
I8 = mybir.dt.int8
F8E4 = mybir.dt.float8e4
F8E5 = mybir.dt.float8e5

KT = D // 128                    # 4 contraction k-tiles
DT = D // 128                    # 4 output d-tiles (= head pairs)
NQB = TQ // 512                  # 2 query blocks of 512
NKB = SKV // 512                 # 4 key blocks of 512
NTK = SKV // 128                 # 16 key tiles of 128
NTM = NTK // 2                   # 8 key-tile pairs (fp8 DoubleRow)
VW = HD + 1                      # 65: V head block width incl. ones column
VWP = 80                         # padded head block (DoubleRow pair stride %16)

LOG2E = 1.4426950408889634
A8 = 4.0 * LOG2E * SCALE         # fast-exp slope for e5m2 bits (SCALE folded)
B8 = 60.0 - 486411.0 / (2 ** 21)  # fast-exp bias (round-to-nearest)

# exp engine assignment: strict alternation - each DoubleRow key-tile pair
# gets one ACT tile and one DVE tile running concurrently
def _dve_tile(gidx: int) -> bool:
    return gidx % 2 == 1


AF = mybir.ActivationFunctionType
OP = mybir.AluOpType
DR = mybir.MatmulPerfMode.DoubleRow


def _build():
    nc = bacc.Bacc(None, target_bir_lowering=False, debug=False)

    xqt = nc.dram_tensor("xqt", [D, TQ], BF16, kind="ExternalInput").ap()
    xkv8 = nc.dram_tensor("xkv8", [128, 2, 2, SKV], F8E4, kind="ExternalInput").ap()
    wqt = nc.dram_tensor("wqt", [D, D], BF16, kind="ExternalInput").ap()
    wk8 = nc.dram_tensor("wk8", [128, 2, 2, D], F8E4, kind="ExternalInput").ap()
    wv8 = nc.dram_tensor("wv8", [128, 2, 2, D], F8E4, kind="ExternalInput").ap()
    wot = nc.dram_tensor("wot", [D, D], BF16, kind="ExternalInput").ap()
    biasc = nc.dram_tensor("biasc", [128, 3 * DT + D], F32, kind="ExternalInput").ap()
    outt = nc.dram_tensor("outt", [D, TQ], BF16, kind="ExternalOutput").ap()

    with tile.TileContext(nc) as tc:
        with (
            tc.tile_pool(name="persist", bufs=1) as pp,
            tc.tile_pool(name="ps", bufs=1, space="PSUM") as psp,
            tc.tile_pool(name="e2p", bufs=6) as ep,
            tc.tile_pool(name="ocp", bufs=4) as ocp,
            tc.tile_pool(name="onp", bufs=3) as onp,
            tc.tile_pool(name="on64p", bufs=3) as on64p,
            tc.tile_pool(name="rbcp", bufs=3) as rbcp,
            tc.tile_pool(name="ytp", bufs=4) as ytp,
        ):
            w_q = pp.tile([128, KT, D], BF16)
            w_k8 = pp.tile([128, 2, 2, D], F8E4)
            w_v8 = pp.tile([128, 2, 2, D], F8E4)
            w_o = pp.tile([128, KT, D], BF16)
            qt = pp.tile([128, DT, TQ], BF16)      # Q^T, becomes x^T
            kt = pp.tile([128, DT, SKV], BF16)     # K^T (scores lhsT)
            # V as fp8, key-tile pairs: [p, m, j, head*VWP + d] (ones col at 64)
            v8 = pp.tile([128, NTM, 2, H * VWP], F8E4)
            xq_s = pp.tile([128, KT, TQ], BF16)
            xkv_s = pp.tile([128, 2, 2, SKV], F8E4)
            bias_s = pp.tile([128, 3 * DT + D], F32)
            bq_s = bias_s[:, 0:DT]
            bk_s = bias_s[:, DT : 2 * DT]
            bo_s = bias_s[:, 2 * DT : 3 * DT]
            bv_s = bias_s[:, 3 * DT :]

            # ---- input DMA: biases + K-projection path first ----
            nc.sync.dma_start(bias_s[:], biasc)
            nc.sync.dma_start(w_k8[:], wk8)
            nc.sync.dma_start(xkv_s[:], xkv8)
            wq_r = wqt.rearrange("(o p) d -> p o d", p=128)
            xq_r = xqt.rearrange("(o p) t -> p o t", p=128)
            for k in range(KT):
                nc.sync.dma_start(w_q[:, k, :], wq_r[:, k, :])
                nc.sync.dma_start(xq_s[:, k, :], xq_r[:, k, :])
            nc.sync.dma_start(w_v8[:], wv8)
            nc.sync.dma_start(w_o[:], wot.rearrange("(o p) d -> p o d", p=128))

            # ones columns of V (col 64 of each VWP-wide head block)
            for m in range(NTM):
                for j in range(2):
                    nc.any.memset(
                        v8[:, m, j, :].rearrange("p (h w) -> p h w", w=VWP)[:, :, HD],
                        1.0,
                    )

            # ---------------- phase 1: projections ----------------
            def q_proj(j, q):
                # one 512-wide q block; evac per block (bias via ACT)
                ps = psp.tile([128, 2, 512], F32, tag="s2", bufs=3, name="pjq")
                for k in range(KT):
                    nc.tensor.matmul(
                        ps[:, 0, :], w_q[:, k, j * 128 : (j + 1) * 128],
                        xq_s[:, k, q * 512 : (q + 1) * 512],
                        start=(k == 0), stop=(k == KT - 1),
                    )
                nc.scalar.activation(
                    qt[:, j, q * 512 : (q + 1) * 512], ps[:, 0, :],
                    AF.Identity, bias=bq_s[:, j : j + 1],
                )

            def k_proj2(j, qp):
                # two 512-wide key blocks into one 2-bank tile, single evac
                ps = psp.tile([128, 2, 512], F32, tag="s2", bufs=3, name="pjk")
                for q in range(2):
                    for kp in range(2):
                        nc.tensor.matmul(
                            ps[:, q, :], w_k8[:, kp, :, j * 128 : (j + 1) * 128],
                            xkv_s[:, kp, :, (2 * qp + q) * 512 : (2 * qp + q + 1) * 512],
                            start=(kp == 0), stop=(kp == 1), perf_mode=DR,
                        )
                nc.scalar.activation(
                    kt[:, j, qp * 1024 : (qp + 1) * 1024],
                    ps[:].rearrange("p a b -> p (a b)"),
                    AF.Identity, bias=bk_s[:, j : j + 1],
                )

            def v_proj(i):
                ps = psp.tile([128, 2, 512], F32, tag="s2", bufs=3, name="pjv")
                for kp in range(2):
                    nc.tensor.matmul(
                        ps[:, 0, :], xkv_s[:, kp, :, i * 128 : (i + 1) * 128],
                        w_v8[:, kp, :, :],
                        start=(kp == 0), stop=(kp == 1), perf_mode=DR,
                    )
                nc.vector.tensor_tensor(
                    v8[:, i // 2, i % 2, :]
                    .rearrange("p (h w) -> p h w", w=VWP)[:, :, 0:HD],
                    ps[:, 0, :].rearrange("p (h w) -> p h w", w=HD),
                    bv_s.rearrange("p (h w) -> p h w", w=HD),
                    OP.add,
                )

            # ---------------- phases 1+2+3 interleaved, qb-outer ----------------
            def phase3(qb, j):
                qsl = slice(qb * 512, (qb + 1) * 512)
                z = psp.tile([128, 2, 512], F32, tag="s2", bufs=3, name="zt")
                for k in range(KT):
                    nc.tensor.matmul(
                        z[:, 0, :], w_o[:, k, j * 128 : (j + 1) * 128],
                        qt[:, k, qsl],
                        start=(k == 0), stop=(k == KT - 1),
                    )
                yt = ytp.tile([128, 512], F32)
                nc.scalar.activation(
                    yt[:], z[:, 0, :], AF.Relu, bias=bo_s[:, j : j + 1]
                )
                yt2 = ytp.tile([128, 512], BF16, tag="y2", name="yt2")
                nc.any.tensor_tensor(yt2[:], yt[:], qt[:, j, qsl], OP.add)
                nc.sync.dma_start(outt[j * 128 : (j + 1) * 128, qsl], yt2[:])

            for qb in range(NQB):
                qsl = slice(qb * 512, (qb + 1) * 512)
                for hp in range(DT):
                    if qb == 0:
                        # emit this head-pair's projections just ahead of its
                        # attention; the scheduler fills PE gaps with them
                        q_proj(hp, 0)
                        for qp in range(2):
                            k_proj2(hp, qp)
                        for i in range(4 * hp, 4 * hp + 4):
                            v_proj(i)
                        if hp == DT - 1:
                            for j in range(DT):
                                q_proj(j, 1)
                    h_e, h_o = 2 * hp, 2 * hp + 1
                    o2 = psp.tile([65, 2, 512], F32, tag="o", bufs=1, name="o2")
                    for m in range(NTM):
                        # e2 pair-tile: [p, j(key subtile), parity, q] fp8e5
                        e2 = ep.tile([128, 2, 2, 512], F8E5)
                        for jj in range(2):
                            i = 2 * m + jj
                            gidx = (qb * DT + hp) * NTK + i
                            isl = slice(i * 128, (i + 1) * 128)
                            s2 = psp.tile(
                                [128, 2, 512], F32, tag="s2", bufs=3, name="s2t"
                            )
                            nc.tensor.matmul(
                                s2[:, 0, :], kt[0:64, hp, isl], qt[0:64, hp, qsl],
                                start=True, stop=True, tile_position=(0, 0),
                            )
                            nc.tensor.matmul(
                                s2[:, 1, :], kt[64:128, hp, isl],
                                qt[64:128, hp, qsl],
                                start=True, stop=True, tile_position=(64, 0),
                            )
                            if _dve_tile(gidx):
                                nc.vector.tensor_scalar(
                                    e2[:, jj, :, :]
                                    .rearrange("p a b -> p (a b)").bitcast(I8),
                                    s2[:].rearrange("p a b -> p (a b)"),
                                    A8, B8, OP.mult, OP.add,
                                )
                            else:
                                nc.scalar.activation(
                                    e2[:, jj, :, :].rearrange("p a b -> p (a b)"),
                                    s2[:].rearrange("p a b -> p (a b)"),
                                    AF.Exp, scale=SCALE,
                                )
                        for p, h in ((0, h_e), (1, h_o)):
                            nc.tensor.matmul(
                                o2[:, p, :],
                                v8[:, m, :, h * VWP : h * VWP + VW],
                                e2[:, :, p, :],
                                start=(m == 0), stop=(m == NTM - 1),
                                perf_mode=DR,
                            )

                    # evacuate o + r; normalize; residual-add into qt
                    oc = ocp.tile([65, 2, 512], F32, name="oct")
                    nc.scalar.activation(oc[:, 0, :], o2[:, 0, :], AF.Copy)
                    nc.vector.tensor_copy(oc[:, 1, :], o2[:, 1, :])
                    # r rows ([1,1024] f32) -> [8,128] tile so the reciprocal
                    # runs 8 lanes wide (DMA reshapes by linearization)
                    rcol = rbcp.tile([8, 128], F32, tag="rc", name="rcol")
                    nc.sync.dma_start(rcol[:], oc[64:65, :, :])
                    rinv = rbcp.tile([8, 128], F32, tag="ri", name="rinv")
                    nc.vector.reciprocal_approx_fast(rinv[:], rcol[:])
                    rflat = rbcp.tile([1, 2, 512], F32, tag="rf", name="rflat")
                    nc.sync.dma_start(rflat[:], rinv[:])
                    rbc = rbcp.tile([64, 2, 512], F32, tag="rb", name="rbc")
                    nc.gpsimd.partition_broadcast(rbc[:, 0, :], rflat[0:1, 0, :])
                    nc.gpsimd.partition_broadcast(rbc[:, 1, :], rflat[0:1, 1, :])
                    on = onp.tile([64, 2, 512], BF16)
                    nc.any.tensor_tensor(on[:], oc[0:64, :, :], rbc[:], OP.mult)
                    nc.any.tensor_tensor(
                        qt[0:64, hp, qsl], qt[0:64, hp, qsl], on[:, 0, :], OP.add
                    )
                    on64 = on64p.tile([128, 512], BF16)
                    nc.sync.dma_start(on64[64:128, :], on[:, 1, :])
                    nc.any.tensor_tensor(
                        qt[64:128, hp, qsl], qt[64:128, hp, qsl],
                        on64[64:128, :], OP.add,
                    )
                    if qb == 1:
                        # overlap qb0's output projection with qb1's attention
                        phase3(0, hp)

            for j in range(DT):
                phase3(1, j)

    nc.compile()
    return nc


_NC = None


def _get_nc():
    global _NC
    if _NC is None:
        _NC = _build()
    return _NC


def _interleave8(a):
    """[512, N] f32 -> [128, 2, 2, N] fp8e4 with row (kp*256 + j*128 + p) at
    [p, kp, j]."""
    f8 = ml_dtypes.float8_e4m3
    return np.ascontiguousarray(
        a.reshape(2, 2, 128, a.shape[1]).transpose(2, 0, 1, 3)
    ).astype(f8)


def kernel(**inputs) -> np.ndarray:
    bf = ml_dtypes.bfloat16
    q = np.asarray(inputs["query"], dtype=np.float32)
    kv = np.asarray(inputs["key_value"], dtype=np.float32)
    shared = {
        "wqt": np.ascontiguousarray(np.asarray(inputs["Wq"], np.float32).T).astype(bf),
        "wk8": _interleave8(np.asarray(inputs["Wk"], np.float32).T),
        "wv8": _interleave8(np.asarray(inputs["Wv"], np.float32).T),
        "wot": np.ascontiguousarray(np.asarray(inputs["Wo"], np.float32).T).astype(bf),
        "biasc": np.ascontiguousarray(np.concatenate(
            [
                np.asarray(inputs["bq"], np.float32).reshape(DT, 128).T,
                np.asarray(inputs["bk"], np.float32).reshape(DT, 128).T,
                np.asarray(inputs["bo"], np.float32).reshape(DT, 128).T,
                np.broadcast_to(np.asarray(inputs["bv"], np.float32), (128, D)),
            ],
            axis=1,
        )),
    }
    in_maps = []
    for c in range(NCORES):
        b, half = divmod(c, 2)
        qs = q[b, half * TQ : (half + 1) * TQ]
        in_maps.append(
            {
                "xqt": np.ascontiguousarray(qs.T).astype(bf),
                "xkv8": _interleave8(np.ascontiguousarray(kv[b].T)),
                **shared,
            }
        )

    nc = _get_nc()
    res = run_bass_kernel_spmd(nc, in_maps, core_ids=list(range(NCORES)))
    kernel._last_results = res  # for test harness introspection

    out = np.empty((B, SQ, D), np.float32)
    for c in range(NCORES):
        b, half = divmod(c, 2)
        out[b, half * TQ : (half + 1) * TQ] = res.results[c]["outt"].astype(np.float32).T
    return out
